# revision 11
# baseline (speedup 1.0000x reference)
"""GCN (gather-scale-segment_max x2) on 8 Trainium2 NeuronCores.

Strategy (2 SPMD launches):
  Edges are sharded by destination-node block (12500 nodes per core), so each
  core owns the complete reduction for its nodes and no cross-core reduce is
  needed. The host only re-orders bytes (index take / pad) into per-slot
  tables; all value-level arithmetic, reductions and matmuls happen on the
  NeuronCores. Per-slot message rows are shipped as bf16 (half the PCIe/HBM
  bytes of f32); the scale product is accumulated into f32 on the vector
  engine, and the two-level segment max, the transpose and the linear layers
  run in f32 exactly as before.
    launch 1: msg*ts, two-level segment max, @W1+b1, relu
    launch 2: msg*ts, two-level segment max, @W2+b2
"""

import os

os.environ.setdefault("JAX_COMPILATION_CACHE_DIR", "/tmp/jax_kernel_cache")

import numpy as np
import ml_dtypes
from concourse import bass, mybir
from concourse.bass_utils import run_bass_kernel_spmd
from concourse.tile import TileContext
from bass_rust import ScopedClock

try:
    import jax
    jax.config.update("jax_compilation_cache_dir", "/tmp/jax_kernel_cache")
    jax.config.update("jax_persistent_cache_min_compile_time_secs", 0.5)
except Exception:
    pass

N_CORES = 8
N_NODES = 100000
B = N_NODES // N_CORES          # 12500 dst nodes per core
P = 128
F1, HID, NCLS = 16, 8, 2
STRIPE_ROWS = 32                 # 32 rows x 16 slots = 512 slots per stripe

_DT = mybir.dt.float32
_BF = mybir.dt.bfloat16


# ---------------------------------------------------------------- tile patch
class _Tc(TileContext):
    """This walrus build allows only ONE sync-wait per instruction; split the
    end-of-kernel drain waits across SP nops."""

    def _drain_and_barrier(self, tick_clock, wait_clock):
        holder = self.nc.sync.nop(nofuse=True, hint="drain_waits")
        wait_clock.add_sem_waits(holder.ins, ScopedClock({None: tick_clock.global_clock}))
        si = holder.ins.sync_info
        waits = list(si.on_wait) if si and si.on_wait else []
        if len(waits) > 1:
            upd = list(si.on_update) if si.on_update else []
            holder.ins.sync_info = mybir.SyncInfo(on_wait=waits[:1], on_update=upd)
            for w in waits[1:]:
                extra = self.nc.sync.nop(nofuse=True, hint="drain_waits")
                extra.ins.sync_info = mybir.SyncInfo(on_wait=[w], on_update=[])
        self.nc.sync.drain()
        self.nc.all_engine_barrier()
        assert self.sems is not None
        popped = self.nc._tile_sem_poison_stack.pop()
        assert popped is self._sem_poison
        self.nc.clear_and_free_semaphores(list(self.sems.allocated().values()))
        self.nc.all_engine_barrier()


def _split_waits(nc, max_waits=1):
    n = 0
    for fn in nc.m.functions:
        for bb in fn.blocks:
            out = []
            for inst in bb.instructions:
                si = inst.sync_info
                waits = list(si.on_wait) if si and si.on_wait else []
                if len(waits) > max_waits:
                    for w in waits[:-max_waits]:
                        n += 1
                        nop = mybir.InstNoOp(name=f"I-ws-{n}")
                        nop.engine = inst.engine
                        nop.sync_info = mybir.SyncInfo(on_wait=[w], on_update=[])
                        out.append(nop)
                    inst.sync_info = mybir.SyncInfo(
                        on_wait=waits[-max_waits:],
                        on_update=list(si.on_update) if si.on_update else [],
                    )
                out.append(inst)
            bb.instructions[:] = out
    return n


class _Shard:
    pass


def _prepare(src, dst, ts):
    """Per-core shards plus cross-core-uniform slot-table structure."""
    shards = []
    blk = dst // B
    for n in range(N_CORES):
        sh = _Shard()
        sel = np.nonzero(blk == n)[0]
        sh.es = src[sel].astype(np.int64)
        sh.ed = (dst[sel] - n * B).astype(np.int64)
        sh.ts = ts[sel]
        sh.En = len(sel)
        sh.order_d = np.argsort(sh.ed, kind="stable").astype(np.int64)
        sh.deg = np.bincount(sh.ed, minlength=B).astype(np.int64)
        assert sh.deg.min() >= 1
        sh.dstart = np.concatenate([[0], np.cumsum(sh.deg)[:-1]])
        sh.rows_i = -(-sh.deg // 16)
        shards.append(sh)

    # --- uniform rowcount-group structure
    rmax = max(int(sh.rows_i.max()) for sh in shards)
    count_r = np.zeros(rmax + 1, np.int64)
    for sh in shards:
        count_r = np.maximum(count_r, np.bincount(sh.rows_i, minlength=rmax + 1))
    count_r[0] = 0
    m_r = -(-count_r // P)          # node-grid rows per rowcount group
    rows_pp = int((m_r * np.arange(rmax + 1)).sum())
    pad_rows = (-rows_pp) % STRIPE_ROWS
    if pad_rows:
        m_r[1] += pad_rows          # dummy single-row nodes to align stripes
        rows_pp += pad_rows
    ROWS_PP = rows_pp
    S_PP = ROWS_PP * 16
    M = int(m_r.sum())
    r_list = [int(r) for r in np.nonzero(m_r)[0]]
    groups = [(r, int(m_r[r])) for r in r_list]

    for sh in shards:
        grids = []
        slot_chunks = []
        for r in r_list:
            nodes = np.nonzero(sh.rows_i == r)[0]
            need = int(m_r[r]) * P
            g = np.full(need, -1, np.int64)
            g[:len(nodes)] = nodes
            g = g.reshape(int(m_r[r]), P)
            grids.append(g)
            gg = np.where(g < 0, 0, g)
            start = sh.dstart[gg][:, :, None]
            degg = sh.deg[gg][:, :, None]
            j = np.arange(16 * r)[None, None, :]
            eidx = sh.order_d[start + np.minimum(j, degg - 1)]
            slot_chunks.append(eidx.transpose(1, 0, 2).reshape(P, -1))
        sh.node_grid = np.concatenate(grids, axis=0)          # [M, P]
        sh.slot_edge = np.concatenate(slot_chunks, axis=1)    # [P, S_PP]
        assert sh.slot_edge.shape == (P, S_PP)

    cfg = dict(GROUPS=groups, ROWS_PP=ROWS_PP, S_PP=S_PP, M=M)
    return shards, cfg


# ------------------------------------------------------------ device build
def _build_reduce_g(cfg, hidden):
    """Layer 1 with on-device gather: x ships once as quad-packed 256B rows
    (xq = x.reshape(25000, 64), f32); per-slot rows are fetched by dma_gather
    with int16 indices src>>2, and the 4-way sub-row select is folded into
    the segment max via an is_equal mask on src&3 (+-1e30 offsets)."""
    feat, KQ, SCOL = F1, 4, 64            # 64 s-columns/chunk = 8192 idxs
    S_PP, ROWS_PP, M = cfg["S_PP"], cfg["ROWS_PP"], cfg["M"]
    n_stripes = ROWS_PP // STRIPE_ROWS
    SS = STRIPE_ROWS * 16
    CH = SCOL * P                          # idxs per gather call
    n_ch = SS // SCOL                      # chunks per stripe
    NQ = N_NODES // KQ

    nc = bass.Bass("TRN2", target_bir_lowering=False, debug=False,
                   num_devices=N_CORES, num_swdge_queues=4)
    IDXC = S_PP * P // 16                  # wrapped idx cols total
    SC_ST = SS * P // 16                   # wrapped idx cols per stripe
    SC_CH = CH // 16                       # wrapped idx cols per call
    xq = nc.declare_dram_parameter("xq", [NQ, KQ * feat], _DT, isOutput=False)
    idxd = nc.declare_dram_parameter("idx", [P, IDXC], mybir.dt.int16,
                                     isOutput=False)
    kkd = nc.declare_dram_parameter("kk", [P, S_PP], _DT, isOutput=False)
    tsd = nc.declare_dram_parameter("ts", [P, S_PP], _DT, isOutput=False)
    wd = nc.declare_dram_parameter("w", [feat, hidden], _DT, isOutput=False)
    bd = nc.declare_dram_parameter("b", [hidden, 1], _DT, isOutput=False)
    ident = nc.declare_dram_parameter("ident", [P, P], _DT, isOutput=False)
    hT = nc.declare_dram_parameter("hT", [hidden, M, P], _DT, isOutput=True)

    with _Tc(nc) as tc:
        from concourse import library_config
        nc.gpsimd.load_library(library_config.mlp)
        creg = nc.gpsimd.to_reg(CH)
        with tc.tile_pool(name="sb", bufs=3) as sb, \
             tc.tile_pool(name="gp", bufs=2) as gp, \
             tc.tile_pool(name="scp", bufs=2) as scp, \
             tc.tile_pool(name="big", bufs=1) as big, \
             tc.tile_pool(name="cst", bufs=1) as cst, \
             tc.tile_pool(name="ps", bufs=2, space="PSUM") as ps:
            w_t = cst.tile([feat, hidden], _DT)
            nc.sync.dma_start(out=w_t[:, :], in_=wd[:, :])
            b_t = cst.tile([hidden, 1], _DT)
            nc.sync.dma_start(out=b_t[:, :], in_=bd[:, :])
            id_t = cst.tile([P, P], _DT)
            nc.sync.dma_start(out=id_t[:, :], in_=ident[:, :])

            part = big.tile([P, ROWS_PP, feat], _DT)
            for st in range(n_stripes):
                s0 = st * SS
                ist = sb.tile([P, SC_ST], mybir.dt.int16, tag="ist")
                nc.sync.dma_start(out=ist[:, :],
                                  in_=idxd[:, st * SC_ST:(st + 1) * SC_ST])
                kk_t = sb.tile([P, SS], _DT, tag="kkt")
                nc.scalar.dma_start(out=kk_t[:, :], in_=kkd[:, s0:s0 + SS])
                ts_t = sb.tile([P, SS], _DT, tag="tst")
                nc.scalar.dma_start(out=ts_t[:, :], in_=tsd[:, s0:s0 + SS])
                tsq = scp.tile([P, SS, KQ], _DT, tag="tsq")
                offq = scp.tile([P, SS, KQ], _DT, tag="offq")
                msk = scp.tile([P, SS], _DT, tag="msk")
                for k in range(KQ):
                    nc.vector.tensor_scalar(out=msk[:, :], in0=kk_t[:, :],
                                            scalar1=float(k), scalar2=0.0,
                                            op0=mybir.AluOpType.is_equal)
                    nc.vector.tensor_tensor(out=tsq[:, :, k], in0=msk[:, :],
                                            in1=ts_t[:, :],
                                            op=mybir.AluOpType.mult)
                    nc.vector.tensor_scalar(out=offq[:, :, k], in0=msk[:, :],
                                            scalar1=1e30, scalar2=-1e30,
                                            op0=mybir.AluOpType.mult,
                                            op1=mybir.AluOpType.add)
                for c in range(n_ch):
                    g = gp.tile([P, SCOL, KQ * feat], _DT, tag="g")
                    nc.gpsimd.dma_gather(g[:, :, :], xq[:, :],
                                         ist[:, c * SC_CH:(c + 1) * SC_CH],
                                         CH, creg, KQ * feat,
                                         transpose=False, single_packet=False,
                                         queue_num=c % 4)
                    cs = c * SCOL
                    gv = g[:, :, :].rearrange("p s (k f) -> p s k f", k=KQ)
                    nc.vector.tensor_tensor(
                        out=gv, in0=gv,
                        in1=offq[:, cs:cs + SCOL, :, None]
                        .to_broadcast([P, SCOL, KQ, feat]),
                        op=mybir.AluOpType.add)
                    nc.vector.tensor_tensor(
                        out=gv, in0=gv,
                        in1=tsq[:, cs:cs + SCOL, :, None]
                        .to_broadcast([P, SCOL, KQ, feat]),
                        op=mybir.AluOpType.mult)
                    sc1 = scp.tile([P, SCOL, feat], _DT, tag="sc1")
                    nc.vector.tensor_reduce(
                        out=sc1[:, :, :],
                        in_=g[:, :, :].rearrange("p s (k f) -> p s f k", k=KQ),
                        axis=mybir.AxisListType.X, op=mybir.AluOpType.max)
                    nc.vector.tensor_reduce(
                        out=part[:, st * STRIPE_ROWS + c * (SCOL // 16):
                                 st * STRIPE_ROWS + (c + 1) * (SCOL // 16), :],
                        in_=sc1[:, :, :].rearrange(
                            "p (rows sp) f -> p rows f sp", sp=16),
                        axis=mybir.AxisListType.X, op=mybir.AluOpType.max)

            agg = big.tile([P, M, feat], _DT)
            row0 = node0 = 0
            for r, m in cfg["GROUPS"]:
                nc.vector.tensor_reduce(
                    out=agg[:, node0:node0 + m, :],
                    in_=part[:, row0:row0 + r * m, :].rearrange(
                        "p (m r) f -> p m f r", r=r),
                    axis=mybir.AxisListType.X, op=mybir.AluOpType.max)
                row0 += r * m
                node0 += m

            for ms in range(M):
                atp = ps.tile([feat, P], _DT, tag="tp")
                nc.tensor.transpose(out=atp[:, :], in_=agg[:, ms, :],
                                    identity=id_t[:, :])
                ats = sb.tile([feat, P], _DT, tag="ats")
                nc.scalar.copy(out=ats[:, :], in_=atp[:, :])
                hp = ps.tile([hidden, P], _DT, tag="hp")
                nc.tensor.matmul(out=hp[:, :], lhsT=w_t[:, :], rhs=ats[:, :],
                                 start=True, stop=True)
                hs = sb.tile([hidden, P], _DT, tag="hs")
                nc.scalar.activation(out=hs[:, :], in_=hp[:, :],
                                     func=mybir.ActivationFunctionType.Relu,
                                     bias=b_t[:, :], scale=1.0)
                nc.sync.dma_start(out=hT[:, ms, :], in_=hs[:, :])
    _split_waits(nc)
    from concourse.library_overlay import lower_extended_insts
    lower_extended_insts(nc)
    return nc


def _build_reduce(cfg, feat, hidden, relu):
    """msg(bf16)*ts(bf16) -> f32 -> two-level segmented max -> (@W + b)
    [-> relu] -> hT."""
    S_PP, ROWS_PP, M = cfg["S_PP"], cfg["ROWS_PP"], cfg["M"]
    n_stripes = ROWS_PP // STRIPE_ROWS
    SS = STRIPE_ROWS * 16

    nc = bass.Bass("TRN2", target_bir_lowering=False, debug=False,
                   num_devices=N_CORES)
    msg = nc.declare_dram_parameter("msg", [P, S_PP, feat], _BF, isOutput=False)
    tsd = nc.declare_dram_parameter("ts", [P, S_PP], _BF, isOutput=False)
    wd = nc.declare_dram_parameter("w", [feat, hidden], _DT, isOutput=False)
    bd = nc.declare_dram_parameter("b", [hidden, 1], _DT, isOutput=False)
    ident = nc.declare_dram_parameter("ident", [P, P], _DT, isOutput=False)
    hT = nc.declare_dram_parameter("hT", [hidden, M, P], _DT, isOutput=True)

    with _Tc(nc) as tc:
        with tc.tile_pool(name="sb", bufs=3) as sb, \
             tc.tile_pool(name="scp", bufs=2) as scp, \
             tc.tile_pool(name="big", bufs=1) as big, \
             tc.tile_pool(name="cst", bufs=1) as cst, \
             tc.tile_pool(name="ps", bufs=2, space="PSUM") as ps:
            w_t = cst.tile([feat, hidden], _DT)
            nc.sync.dma_start(out=w_t[:, :], in_=wd[:, :])
            b_t = cst.tile([hidden, 1], _DT)
            nc.sync.dma_start(out=b_t[:, :], in_=bd[:, :])
            id_t = cst.tile([P, P], _DT)
            nc.sync.dma_start(out=id_t[:, :], in_=ident[:, :])

            part = big.tile([P, ROWS_PP, feat], _DT)
            for st in range(n_stripes):
                mt = sb.tile([P, SS, feat], _BF, tag="mt")
                nc.sync.dma_start(out=mt[:, :, :],
                                  in_=msg[:, st * SS:(st + 1) * SS, :])
                tt = sb.tile([P, SS], _BF, tag="tt")
                nc.scalar.dma_start(out=tt[:, :],
                                    in_=tsd[:, st * SS:(st + 1) * SS])
                sc = scp.tile([P, SS, feat], _DT, tag="sc")
                nc.vector.tensor_tensor(
                    out=sc[:, :, :], in0=mt[:, :, :],
                    in1=tt[:, :, None].to_broadcast([P, SS, feat]),
                    op=mybir.AluOpType.mult)
                nc.vector.tensor_reduce(
                    out=part[:, st * STRIPE_ROWS:(st + 1) * STRIPE_ROWS, :],
                    in_=sc[:, :, :].rearrange("p (rows s) f -> p rows f s", s=16),
                    axis=mybir.AxisListType.X, op=mybir.AluOpType.max)

            agg = big.tile([P, M, feat], _DT)
            row0 = node0 = 0
            for r, m in cfg["GROUPS"]:
                nc.vector.tensor_reduce(
                    out=agg[:, node0:node0 + m, :],
                    in_=part[:, row0:row0 + r * m, :].rearrange(
                        "p (m r) f -> p m f r", r=r),
                    axis=mybir.AxisListType.X, op=mybir.AluOpType.max)
                row0 += r * m
                node0 += m

            func = (mybir.ActivationFunctionType.Relu if relu
                    else mybir.ActivationFunctionType.Identity)
            for ms in range(M):
                atp = ps.tile([feat, P], _DT, tag="tp")
                nc.tensor.transpose(out=atp[:, :], in_=agg[:, ms, :],
                                    identity=id_t[:, :])
                ats = sb.tile([feat, P], _DT, tag="ats")
                nc.scalar.copy(out=ats[:, :], in_=atp[:, :])
                hp = ps.tile([hidden, P], _DT, tag="hp")
                nc.tensor.matmul(out=hp[:, :], lhsT=w_t[:, :], rhs=ats[:, :],
                                 start=True, stop=True)
                hs = sb.tile([hidden, P], _DT, tag="hs")
                nc.scalar.activation(out=hs[:, :], in_=hp[:, :], func=func,
                                     bias=b_t[:, :], scale=1.0)
                nc.sync.dma_start(out=hT[:, ms, :], in_=hs[:, :])
    _split_waits(nc)
    return nc


# ------------------------------------------------------------------- kernel
_CACHE = {}
LAST_TIMINGS = {}


def kernel(x, src, dst, timestamp, W1, b1, W2, b2):
    x = np.ascontiguousarray(np.asarray(x, np.float32))
    src = np.asarray(src, np.int32)
    dst = np.asarray(dst, np.int32)
    timestamp = np.asarray(timestamp, np.float32)
    W1 = np.asarray(W1, np.float32)
    b1 = np.asarray(b1, np.float32)
    W2 = np.asarray(W2, np.float32)
    b2 = np.asarray(b2, np.float32)

    shards, cfg = _prepare(src, dst, timestamp)
    M, S_PP = cfg["M"], cfg["S_PP"]
    identv = np.eye(P, dtype=np.float32)
    cores = list(range(N_CORES))

    import time as _time

    # ---- launch 1: on-device gather + scale + segment max + linear1 + relu
    # x ships once as quad-packed rows (pure reshape); slot tables are int16
    # indices + sub-row selectors + timestamps (host re-orders bytes only)
    xqv = np.ascontiguousarray(x.reshape(N_NODES // 4, 64))
    in1 = []
    for n, sh in enumerate(shards):
        sos = sh.es[sh.slot_edge]                       # [P, S_PP]
        flat = (sos.T.ravel() >> 2).astype(np.int16)    # s-major gather order
        w16 = flat.reshape(-1, 512, 16).transpose(0, 2, 1)  # per-call wrap
        wrapped = np.tile(np.concatenate(list(w16), axis=1), (8, 1))
        in1.append({
            "xq": xqv,
            "idx": np.ascontiguousarray(wrapped),
            "kk": np.ascontiguousarray((sos & 3).astype(np.float32)),
            "ts": np.ascontiguousarray(sh.ts[sh.slot_edge]),
            "w": W1, "b": np.ascontiguousarray(b1[:, None]),
            "ident": identv,
        })
    nc1 = _build_reduce_g(cfg, HID)
    _t = _time.time()
    r1 = run_bass_kernel_spmd(nc1, in1, cores).results
    LAST_TIMINGS["reduce_1"] = _time.time() - _t

    h_full = np.zeros((N_NODES, HID), np.float32)
    for n, sh in enumerate(shards):
        hT = r1[n]["hT"]                               # [HID, M, P]
        hb = hT.transpose(1, 2, 0)                     # [M, P, HID]
        valid = sh.node_grid >= 0
        h_full[n * B + sh.node_grid[valid]] = hb[valid]
    h_bf = h_full.astype(ml_dtypes.bfloat16)

    # ---- launch 2: scale + segment max + linear2
    in2 = []
    for n, sh in enumerate(shards):
        src_of_slot = sh.es[sh.slot_edge]
        in2.append({
            "msg": np.ascontiguousarray(h_bf[src_of_slot]),
            "ts": np.ascontiguousarray(
                sh.ts[sh.slot_edge].astype(ml_dtypes.bfloat16)),
            "w": W2, "b": np.ascontiguousarray(b2[:, None]),
            "ident": identv,
        })
    nc2 = _build_reduce(cfg, HID, NCLS, relu=False)
    _t = _time.time()
    r2 = run_bass_kernel_spmd(nc2, in2, cores).results
    LAST_TIMINGS["reduce_2"] = _time.time() - _t

    out = np.zeros((N_NODES, NCLS), np.float32)
    for n, sh in enumerate(shards):
        oT = r2[n]["hT"]
        ob = oT.transpose(1, 2, 0)
        valid = sh.node_grid >= 0
        out[n * B + sh.node_grid[valid]] = ob[valid]
    return out


# revision 14
# speedup vs baseline: 2.3830x; 2.3830x over previous
"""GCN (gather-scale-segment_max x2) on 8 Trainium2 NeuronCores.

Strategy (2 SPMD launches):
  Edges are sharded by destination-node block (12500 nodes per core), so each
  core owns the complete reduction for its nodes and no cross-core reduce is
  needed. The host only re-orders bytes (index take / pad) into per-slot
  tables; all value-level arithmetic, reductions and matmuls happen on the
  NeuronCores. Per-slot message rows are shipped as bf16 (half the PCIe/HBM
  bytes of f32); the scale product is accumulated into f32 on the vector
  engine, and the two-level segment max, the transpose and the linear layers
  run in f32 exactly as before.
    launch 1: msg*ts, two-level segment max, @W1+b1, relu
    launch 2: msg*ts, two-level segment max, @W2+b2
"""

import os

os.environ.setdefault("JAX_COMPILATION_CACHE_DIR", "/tmp/jax_kernel_cache")

import numpy as np
import ml_dtypes
from concourse import bass, mybir
from concourse.bass_utils import run_bass_kernel_spmd
from concourse.tile import TileContext
from bass_rust import ScopedClock

try:
    import jax
    jax.config.update("jax_compilation_cache_dir", "/tmp/jax_kernel_cache")
    jax.config.update("jax_persistent_cache_min_compile_time_secs", 0.5)
except Exception:
    pass

N_CORES = 8
N_NODES = 100000
B = N_NODES // N_CORES          # 12500 dst nodes per core
P = 128
F1, HID, NCLS = 16, 8, 2
STRIPE_ROWS = 32                 # 32 rows x 16 slots = 512 slots per stripe

_DT = mybir.dt.float32
_BF = mybir.dt.bfloat16


# ---------------------------------------------------------------- tile patch
class _Tc(TileContext):
    """This walrus build allows only ONE sync-wait per instruction; split the
    end-of-kernel drain waits across SP nops."""

    def _drain_and_barrier(self, tick_clock, wait_clock):
        holder = self.nc.sync.nop(nofuse=True, hint="drain_waits")
        wait_clock.add_sem_waits(holder.ins, ScopedClock({None: tick_clock.global_clock}))
        si = holder.ins.sync_info
        waits = list(si.on_wait) if si and si.on_wait else []
        if len(waits) > 1:
            upd = list(si.on_update) if si.on_update else []
            holder.ins.sync_info = mybir.SyncInfo(on_wait=waits[:1], on_update=upd)
            for w in waits[1:]:
                extra = self.nc.sync.nop(nofuse=True, hint="drain_waits")
                extra.ins.sync_info = mybir.SyncInfo(on_wait=[w], on_update=[])
        self.nc.sync.drain()
        self.nc.all_engine_barrier()
        assert self.sems is not None
        popped = self.nc._tile_sem_poison_stack.pop()
        assert popped is self._sem_poison
        self.nc.clear_and_free_semaphores(list(self.sems.allocated().values()))
        self.nc.all_engine_barrier()


def _split_waits(nc, max_waits=1):
    n = 0
    for fn in nc.m.functions:
        for bb in fn.blocks:
            out = []
            for inst in bb.instructions:
                si = inst.sync_info
                waits = list(si.on_wait) if si and si.on_wait else []
                if len(waits) > max_waits:
                    for w in waits[:-max_waits]:
                        n += 1
                        nop = mybir.InstNoOp(name=f"I-ws-{n}")
                        nop.engine = inst.engine
                        nop.sync_info = mybir.SyncInfo(on_wait=[w], on_update=[])
                        out.append(nop)
                    inst.sync_info = mybir.SyncInfo(
                        on_wait=waits[-max_waits:],
                        on_update=list(si.on_update) if si.on_update else [],
                    )
                out.append(inst)
            bb.instructions[:] = out
    return n


class _Shard:
    pass


def _prepare(src, dst, ts):
    """Per-core shards plus cross-core-uniform slot-table structure."""
    shards = []
    blk = dst // B
    for n in range(N_CORES):
        sh = _Shard()
        sel = np.nonzero(blk == n)[0]
        sh.es = src[sel].astype(np.int64)
        sh.ed = (dst[sel] - n * B).astype(np.int64)
        sh.ts = ts[sel]
        sh.En = len(sel)
        sh.order_d = np.argsort(sh.ed, kind="stable").astype(np.int64)
        sh.deg = np.bincount(sh.ed, minlength=B).astype(np.int64)
        assert sh.deg.min() >= 1
        sh.dstart = np.concatenate([[0], np.cumsum(sh.deg)[:-1]])
        sh.rows_i = -(-sh.deg // 16)
        shards.append(sh)

    # --- uniform rowcount-group structure
    rmax = max(int(sh.rows_i.max()) for sh in shards)
    count_r = np.zeros(rmax + 1, np.int64)
    for sh in shards:
        count_r = np.maximum(count_r, np.bincount(sh.rows_i, minlength=rmax + 1))
    count_r[0] = 0
    m_r = -(-count_r // P)          # node-grid rows per rowcount group
    rows_pp = int((m_r * np.arange(rmax + 1)).sum())
    pad_rows = (-rows_pp) % STRIPE_ROWS
    if pad_rows:
        m_r[1] += pad_rows          # dummy single-row nodes to align stripes
        rows_pp += pad_rows
    ROWS_PP = rows_pp
    S_PP = ROWS_PP * 16
    M = int(m_r.sum())
    r_list = [int(r) for r in np.nonzero(m_r)[0]]
    groups = [(r, int(m_r[r])) for r in r_list]

    for sh in shards:
        grids = []
        slot_chunks = []
        for r in r_list:
            nodes = np.nonzero(sh.rows_i == r)[0]
            need = int(m_r[r]) * P
            g = np.full(need, -1, np.int64)
            g[:len(nodes)] = nodes
            g = g.reshape(int(m_r[r]), P)
            grids.append(g)
            gg = np.where(g < 0, 0, g)
            start = sh.dstart[gg][:, :, None]
            degg = sh.deg[gg][:, :, None]
            j = np.arange(16 * r)[None, None, :]
            eidx = sh.order_d[start + np.minimum(j, degg - 1)]
            slot_chunks.append(eidx.transpose(1, 0, 2).reshape(P, -1))
        sh.node_grid = np.concatenate(grids, axis=0)          # [M, P]
        sh.slot_edge = np.concatenate(slot_chunks, axis=1)    # [P, S_PP]
        assert sh.slot_edge.shape == (P, S_PP)

    cfg = dict(GROUPS=groups, ROWS_PP=ROWS_PP, S_PP=S_PP, M=M)
    return shards, cfg


# ------------------------------------------------------------ device build
def _build_reduce_g(cfg, hidden):
    """Layer 1 with on-device gather: x ships once as quad-packed 256B rows
    (xq = x.reshape(25000, 64), f32); per-slot rows are fetched by dma_gather
    with int16 indices src>>2, and the 4-way sub-row select is folded into
    the segment max via an is_equal mask on src&3 (+-1e30 offsets)."""
    feat, KQ, SCOL = F1, 4, 64            # 64 s-columns/chunk = 8192 idxs
    S_PP, ROWS_PP, M = cfg["S_PP"], cfg["ROWS_PP"], cfg["M"]
    n_stripes = ROWS_PP // STRIPE_ROWS
    SS = STRIPE_ROWS * 16
    CH = SCOL * P                          # idxs per gather call
    n_ch = SS // SCOL                      # chunks per stripe
    NQ = N_NODES // KQ

    nc = bass.Bass("TRN2", target_bir_lowering=False, debug=False,
                   num_devices=N_CORES, num_swdge_queues=4)
    IDXC = S_PP * P // 16                  # wrapped idx cols total
    SC_ST = SS * P // 16                   # wrapped idx cols per stripe
    SC_CH = CH // 16                       # wrapped idx cols per call
    xq = nc.declare_dram_parameter("xq", [NQ, KQ * feat], _DT, isOutput=False)
    idxd = nc.declare_dram_parameter("idx", [P, IDXC], mybir.dt.int16,
                                     isOutput=False)
    kkd = nc.declare_dram_parameter("kk", [P, S_PP], _DT, isOutput=False)
    tsd = nc.declare_dram_parameter("ts", [P, S_PP], _DT, isOutput=False)
    wd = nc.declare_dram_parameter("w", [feat, hidden], _DT, isOutput=False)
    bd = nc.declare_dram_parameter("b", [hidden, 1], _DT, isOutput=False)
    ident = nc.declare_dram_parameter("ident", [P, P], _DT, isOutput=False)
    hT = nc.declare_dram_parameter("hT", [hidden, M, P], _DT, isOutput=True)

    with _Tc(nc) as tc:
        from concourse import library_config
        nc.gpsimd.load_library(library_config.mlp)
        creg = nc.gpsimd.to_reg(CH)
        with tc.tile_pool(name="sb", bufs=3) as sb, \
             tc.tile_pool(name="gp", bufs=2) as gp, \
             tc.tile_pool(name="scp", bufs=2) as scp, \
             tc.tile_pool(name="big", bufs=1) as big, \
             tc.tile_pool(name="cst", bufs=1) as cst, \
             tc.tile_pool(name="ps", bufs=2, space="PSUM") as ps:
            w_t = cst.tile([feat, hidden], _DT)
            nc.sync.dma_start(out=w_t[:, :], in_=wd[:, :])
            b_t = cst.tile([hidden, 1], _DT)
            nc.sync.dma_start(out=b_t[:, :], in_=bd[:, :])
            id_t = cst.tile([P, P], _DT)
            nc.sync.dma_start(out=id_t[:, :], in_=ident[:, :])

            part = big.tile([P, ROWS_PP, feat], _DT)
            for st in range(n_stripes):
                s0 = st * SS
                ist = sb.tile([P, SC_ST], mybir.dt.int16, tag="ist")
                nc.sync.dma_start(out=ist[:, :],
                                  in_=idxd[:, st * SC_ST:(st + 1) * SC_ST])
                kk_t = sb.tile([P, SS], _DT, tag="kkt")
                nc.scalar.dma_start(out=kk_t[:, :], in_=kkd[:, s0:s0 + SS])
                ts_t = sb.tile([P, SS], _DT, tag="tst")
                nc.scalar.dma_start(out=ts_t[:, :], in_=tsd[:, s0:s0 + SS])
                tsq = scp.tile([P, SS, KQ], _DT, tag="tsq")
                offq = scp.tile([P, SS, KQ], _DT, tag="offq")
                msk = scp.tile([P, SS], _DT, tag="msk")
                for k in range(KQ):
                    nc.vector.tensor_scalar(out=msk[:, :], in0=kk_t[:, :],
                                            scalar1=float(k), scalar2=0.0,
                                            op0=mybir.AluOpType.is_equal)
                    nc.vector.tensor_tensor(out=tsq[:, :, k], in0=msk[:, :],
                                            in1=ts_t[:, :],
                                            op=mybir.AluOpType.mult)
                    nc.vector.tensor_scalar(out=offq[:, :, k], in0=msk[:, :],
                                            scalar1=1e30, scalar2=-1e30,
                                            op0=mybir.AluOpType.mult,
                                            op1=mybir.AluOpType.add)
                for c in range(n_ch):
                    g = gp.tile([P, SCOL, KQ * feat], _DT, tag="g")
                    nc.gpsimd.dma_gather(g[:, :, :], xq[:, :],
                                         ist[:, c * SC_CH:(c + 1) * SC_CH],
                                         CH, creg, KQ * feat,
                                         transpose=False, single_packet=False,
                                         queue_num=c % 4)
                    cs = c * SCOL
                    gv = g[:, :, :].rearrange("p s (k f) -> p s k f", k=KQ)
                    nc.vector.tensor_tensor(
                        out=gv, in0=gv,
                        in1=offq[:, cs:cs + SCOL, :, None]
                        .to_broadcast([P, SCOL, KQ, feat]),
                        op=mybir.AluOpType.add)
                    nc.vector.tensor_tensor(
                        out=gv, in0=gv,
                        in1=tsq[:, cs:cs + SCOL, :, None]
                        .to_broadcast([P, SCOL, KQ, feat]),
                        op=mybir.AluOpType.mult)
                    sc1 = scp.tile([P, SCOL, feat], _DT, tag="sc1")
                    nc.vector.tensor_reduce(
                        out=sc1[:, :, :],
                        in_=g[:, :, :].rearrange("p s (k f) -> p s f k", k=KQ),
                        axis=mybir.AxisListType.X, op=mybir.AluOpType.max)
                    nc.vector.tensor_reduce(
                        out=part[:, st * STRIPE_ROWS + c * (SCOL // 16):
                                 st * STRIPE_ROWS + (c + 1) * (SCOL // 16), :],
                        in_=sc1[:, :, :].rearrange(
                            "p (rows sp) f -> p rows f sp", sp=16),
                        axis=mybir.AxisListType.X, op=mybir.AluOpType.max)

            agg = big.tile([P, M, feat], _DT)
            row0 = node0 = 0
            for r, m in cfg["GROUPS"]:
                nc.vector.tensor_reduce(
                    out=agg[:, node0:node0 + m, :],
                    in_=part[:, row0:row0 + r * m, :].rearrange(
                        "p (m r) f -> p m f r", r=r),
                    axis=mybir.AxisListType.X, op=mybir.AluOpType.max)
                row0 += r * m
                node0 += m

            for ms in range(M):
                atp = ps.tile([feat, P], _DT, tag="tp")
                nc.tensor.transpose(out=atp[:, :], in_=agg[:, ms, :],
                                    identity=id_t[:, :])
                ats = sb.tile([feat, P], _DT, tag="ats")
                nc.scalar.copy(out=ats[:, :], in_=atp[:, :])
                hp = ps.tile([hidden, P], _DT, tag="hp")
                nc.tensor.matmul(out=hp[:, :], lhsT=w_t[:, :], rhs=ats[:, :],
                                 start=True, stop=True)
                hs = sb.tile([hidden, P], _DT, tag="hs")
                nc.scalar.activation(out=hs[:, :], in_=hp[:, :],
                                     func=mybir.ActivationFunctionType.Relu,
                                     bias=b_t[:, :], scale=1.0)
                nc.sync.dma_start(out=hT[:, ms, :], in_=hs[:, :])
    _split_waits(nc)
    from concourse.library_overlay import lower_extended_insts
    lower_extended_insts(nc)
    return nc


def _build_reduce(cfg, feat, hidden, relu):
    """msg(bf16)*ts(bf16) -> f32 -> two-level segmented max -> (@W + b)
    [-> relu] -> hT."""
    S_PP, ROWS_PP, M = cfg["S_PP"], cfg["ROWS_PP"], cfg["M"]
    n_stripes = ROWS_PP // STRIPE_ROWS
    SS = STRIPE_ROWS * 16

    nc = bass.Bass("TRN2", target_bir_lowering=False, debug=False,
                   num_devices=N_CORES)
    msg = nc.declare_dram_parameter("msg", [P, S_PP, feat], _BF, isOutput=False)
    tsd = nc.declare_dram_parameter("ts", [P, S_PP], _BF, isOutput=False)
    wd = nc.declare_dram_parameter("w", [feat, hidden], _DT, isOutput=False)
    bd = nc.declare_dram_parameter("b", [hidden, 1], _DT, isOutput=False)
    ident = nc.declare_dram_parameter("ident", [P, P], _DT, isOutput=False)
    hT = nc.declare_dram_parameter("hT", [hidden, M, P], _DT, isOutput=True)

    with _Tc(nc) as tc:
        with tc.tile_pool(name="sb", bufs=3) as sb, \
             tc.tile_pool(name="scp", bufs=2) as scp, \
             tc.tile_pool(name="big", bufs=1) as big, \
             tc.tile_pool(name="cst", bufs=1) as cst, \
             tc.tile_pool(name="ps", bufs=2, space="PSUM") as ps:
            w_t = cst.tile([feat, hidden], _DT)
            nc.sync.dma_start(out=w_t[:, :], in_=wd[:, :])
            b_t = cst.tile([hidden, 1], _DT)
            nc.sync.dma_start(out=b_t[:, :], in_=bd[:, :])
            id_t = cst.tile([P, P], _DT)
            nc.sync.dma_start(out=id_t[:, :], in_=ident[:, :])

            part = big.tile([P, ROWS_PP, feat], _DT)
            for st in range(n_stripes):
                mt = sb.tile([P, SS, feat], _BF, tag="mt")
                nc.sync.dma_start(out=mt[:, :, :],
                                  in_=msg[:, st * SS:(st + 1) * SS, :])
                tt = sb.tile([P, SS], _BF, tag="tt")
                nc.scalar.dma_start(out=tt[:, :],
                                    in_=tsd[:, st * SS:(st + 1) * SS])
                sc = scp.tile([P, SS, feat], _DT, tag="sc")
                nc.vector.tensor_tensor(
                    out=sc[:, :, :], in0=mt[:, :, :],
                    in1=tt[:, :, None].to_broadcast([P, SS, feat]),
                    op=mybir.AluOpType.mult)
                nc.vector.tensor_reduce(
                    out=part[:, st * STRIPE_ROWS:(st + 1) * STRIPE_ROWS, :],
                    in_=sc[:, :, :].rearrange("p (rows s) f -> p rows f s", s=16),
                    axis=mybir.AxisListType.X, op=mybir.AluOpType.max)

            agg = big.tile([P, M, feat], _DT)
            row0 = node0 = 0
            for r, m in cfg["GROUPS"]:
                nc.vector.tensor_reduce(
                    out=agg[:, node0:node0 + m, :],
                    in_=part[:, row0:row0 + r * m, :].rearrange(
                        "p (m r) f -> p m f r", r=r),
                    axis=mybir.AxisListType.X, op=mybir.AluOpType.max)
                row0 += r * m
                node0 += m

            func = (mybir.ActivationFunctionType.Relu if relu
                    else mybir.ActivationFunctionType.Identity)
            for ms in range(M):
                atp = ps.tile([feat, P], _DT, tag="tp")
                nc.tensor.transpose(out=atp[:, :], in_=agg[:, ms, :],
                                    identity=id_t[:, :])
                ats = sb.tile([feat, P], _DT, tag="ats")
                nc.scalar.copy(out=ats[:, :], in_=atp[:, :])
                hp = ps.tile([hidden, P], _DT, tag="hp")
                nc.tensor.matmul(out=hp[:, :], lhsT=w_t[:, :], rhs=ats[:, :],
                                 start=True, stop=True)
                hs = sb.tile([hidden, P], _DT, tag="hs")
                nc.scalar.activation(out=hs[:, :], in_=hp[:, :], func=func,
                                     bias=b_t[:, :], scale=1.0)
                nc.sync.dma_start(out=hT[:, ms, :], in_=hs[:, :])
    _split_waits(nc)
    return nc


# ------------------------------------------------------------------- kernel
_CACHE = {}
LAST_TIMINGS = {}


def kernel(x, src, dst, timestamp, W1, b1, W2, b2):
    x = np.ascontiguousarray(np.asarray(x, np.float32))
    src = np.asarray(src, np.int32)
    dst = np.asarray(dst, np.int32)
    timestamp = np.asarray(timestamp, np.float32)
    W1 = np.asarray(W1, np.float32)
    b1 = np.asarray(b1, np.float32)
    W2 = np.asarray(W2, np.float32)
    b2 = np.asarray(b2, np.float32)

    shards, cfg = _prepare(src, dst, timestamp)
    M, S_PP = cfg["M"], cfg["S_PP"]
    identv = np.eye(P, dtype=np.float32)
    cores = list(range(N_CORES))

    import time as _time

    # ---- launch 1: on-device gather + scale + segment max + linear1 + relu
    # x ships once as quad-packed rows (pure reshape); slot tables are int16
    # indices + sub-row selectors + timestamps (host re-orders bytes only)
    xqv = np.ascontiguousarray(x.reshape(N_NODES // 4, 64))
    in1 = []
    for n, sh in enumerate(shards):
        sos = sh.es[sh.slot_edge]                       # [P, S_PP]
        flat = (sos.T.ravel() >> 2).astype(np.int16)    # s-major gather order
        w16 = flat.reshape(-1, 512, 16).transpose(0, 2, 1)  # per-call wrap
        wrapped = np.tile(np.concatenate(list(w16), axis=1), (8, 1))
        in1.append({
            "xq": xqv,
            "idx": np.ascontiguousarray(wrapped),
            "kk": np.ascontiguousarray((sos & 3).astype(np.float32)),
            "ts": np.ascontiguousarray(sh.ts[sh.slot_edge]),
            "w": W1, "b": np.ascontiguousarray(b1[:, None]),
            "ident": identv,
        })
    nc1 = _build_reduce_g(cfg, HID)
    _t = _time.time()
    r1 = run_bass_kernel_spmd(nc1, in1, cores).results
    LAST_TIMINGS["reduce_1"] = _time.time() - _t

    h_full = np.zeros((N_NODES, HID), np.float32)
    for n, sh in enumerate(shards):
        hT = r1[n]["hT"]                               # [HID, M, P]
        hb = hT.transpose(1, 2, 0)                     # [M, P, HID]
        valid = sh.node_grid >= 0
        h_full[n * B + sh.node_grid[valid]] = hb[valid]
    h_bf = h_full.astype(ml_dtypes.bfloat16)

    # ---- launch 2: scale + segment max + linear2
    in2 = []
    for n, sh in enumerate(shards):
        src_of_slot = sh.es[sh.slot_edge]
        in2.append({
            "msg": np.ascontiguousarray(h_bf[src_of_slot]),
            "ts": np.ascontiguousarray(
                sh.ts[sh.slot_edge].astype(ml_dtypes.bfloat16)),
            "w": W2, "b": np.ascontiguousarray(b2[:, None]),
            "ident": identv,
        })
    nc2 = _build_reduce(cfg, HID, NCLS, relu=False)
    _t = _time.time()
    r2 = run_bass_kernel_spmd(nc2, in2, cores).results
    LAST_TIMINGS["reduce_2"] = _time.time() - _t

    out = np.zeros((N_NODES, NCLS), np.float32)
    for n, sh in enumerate(shards):
        oT = r2[n]["hT"]
        ob = oT.transpose(1, 2, 0)
        valid = sh.node_grid >= 0
        out[n * B + sh.node_grid[valid]] = ob[valid]
    return out
